# revision 29
# baseline (speedup 1.0000x reference)
"""ArcFace head on 8 TRN2 NeuronCores.

Class-parallel sharding: core c owns classes [c*12500, (c+1)*12500).
Each core computes out[b, c_local] = S * (F_hat_b . W_hat_c) for its class
shard, and fixes up the target column of rows whose label it owns with the
ArcFace margin via an indirect-DMA scatter.

Host-side prep is layout/indexing only: shard + transpose + bf16-cast of the
weight, a gather of weight rows by label (wg), and flat scatter indices. All
FLOPs (norms, matmul, margin trig) run on device.
"""

import math
import os

import numpy as np

B = 512
D = 512
C = 100000
NCORES = 8
CS = C // NCORES  # 12500 classes per core

M_MARGIN = 0.5
S_SCALE = 64.0
TH = math.cos(math.pi - M_MARGIN)
MM_ = math.sin(math.pi - M_MARGIN) * M_MARGIN
EPS = 1e-12

P = 128
NB = B // P            # 4 b-chunks
NK = D // P            # 4 k-chunks
GW = 512               # c-group width (psum free dim)
NG = (CS + GW - 1) // GW          # 25 groups, last = 212 wide
SGG = 5                # groups per super-group (DMA/square granularity)
NSG = (NG + SGG - 1) // SGG       # 5 super-groups

OOB = 2 ** 28          # scatter index sentinel for unowned rows

_CACHE = {}


def _group_w(g):
    return min(GW, CS - g * GW)


def _sg_bounds(sg):
    lo = sg * SGG * GW
    hi = min(CS, (sg + 1) * SGG * GW)
    return lo, hi


def _build_nc(opts=None):
    opts = opts or {}
    import concourse.tile as tile
    from concourse import bacc, mybir
    import concourse.bass as bass

    dt = mybir.dt
    Alu = mybir.AluOpType
    Act = mybir.ActivationFunctionType

    nc = bacc.Bacc("TRN2", target_bir_lowering=False, debug=False,
                   enable_asserts=False, num_devices=NCORES)

    wt = nc.dram_tensor("wt", [D, CS], dt.bfloat16, kind="ExternalInput").ap()
    feat = nc.dram_tensor("feat", [B, D], dt.float32, kind="ExternalInput").ap()
    featT = nc.dram_tensor("featT", [D, B], dt.float32, kind="ExternalInput").ap()
    wg = nc.dram_tensor("wg", [B, D], dt.float32, kind="ExternalInput").ap()
    sidx = nc.dram_tensor("sidx", [P, NB], dt.int32, kind="ExternalInput").ap()
    outs = [nc.dram_tensor(f"out{j}", [P, CS], dt.bfloat16,
                           kind="ExternalOutput").ap() for j in range(NB)]
    out_flats = [o.rearrange("b c -> (b c)")[:, None] for o in outs]
    host_scatter = bool(opts.get("host_scatter"))
    if host_scatter:
        vout = nc.dram_tensor("vout", [P, NB], dt.float32,
                              kind="ExternalOutput").ap()

    LN_S = math.log(S_SCALE)

    with tile.TileContext(nc) as tc:
        with (
            tc.tile_pool(name="const", bufs=1) as constp,
            tc.tile_pool(name="ph0", bufs=2) as ph0p,
            tc.tile_pool(name="wtp", bufs=8) as wtp,
            tc.tile_pool(name="sqp", bufs=8) as sqp,
            tc.tile_pool(name="invp", bufs=3) as invp,
            tc.tile_pool(name="outp", bufs=6) as outp,
            tc.tile_pool(name="ps_o", bufs=opts.get("ps_o", 6),
                         space="PSUM") as ps_o,
            tc.tile_pool(name="ps_n", bufs=opts.get("ps_n", 2),
                         space="PSUM") as ps_n,
        ):
            # ---- constants ----
            ones_sq = constp.tile([P, P], dt.bfloat16, tag="ones_sq")
            nc.vector.memset(ones_sq[:], 1.0)

            # ---- prefetch + square super-group 0, per-group pieces ----
            wt_tiles = [None] * NK
            sq_tiles = [None] * NK
            sg0_lo, sg0_hi = _sg_bounds(0)
            wtb = opts.get("wtbufs", 2)
            for k in range(NK):
                wt_tiles[k] = wtp.tile([P, SGG * GW], dt.bfloat16,
                                       tag=f"wt{k}", bufs=wtb, name=f"wt_t{k}")
                sq_tiles[k] = sqp.tile([P, SGG * GW], dt.bfloat16,
                                       tag=f"sq{k}", bufs=2, name=f"sq_t{k}")
            for gi in range(1):
                glo = gi * GW
                ghi = min(sg0_hi - sg0_lo, (gi + 1) * GW)
                for k in range(NK):
                    nc.sync.dma_start(
                        wt_tiles[k][:, glo:ghi],
                        wt[k * P:(k + 1) * P, sg0_lo + glo:sg0_lo + ghi])
                    nc.vector.tensor_tensor(
                        out=sq_tiles[k][:, glo:ghi], in0=wt_tiles[k][:, glo:ghi],
                        in1=wt_tiles[k][:, glo:ghi], op=Alu.mult)

            # ---- feature chain first: it gates the main matmuls ----
            fn = []
            fnT = []
            for k in range(NK):
                fnT_k = constp.tile([P, B], dt.bfloat16, tag=f"fnT{k}")
                fnT.append(fnT_k)
            f_ts = []
            wg_ts = []
            invfS = []  # S / |F_b| per b-chunk, folded into the out copy
            for k in range(NK):
                ft_f32 = ph0p.tile([P, B], dt.float32, tag=f"ftf{k}")
                nc.sync.dma_start(ft_f32[:], featT[k * P:(k + 1) * P, :])
                nc.vector.tensor_copy(fnT[k][:], ft_f32[:])  # f32 -> bf16
            for j in range(NB):
                f_t = ph0p.tile([P, D], dt.float32, tag=f"f{j}")
                nc.sync.dma_start(f_t[:], feat[j * P:(j + 1) * P, :])
                f_ts.append(f_t)
            for j in range(NB):
                f_t = f_ts[j]
                fn_t = ph0p.tile([P, D], dt.bfloat16, tag=f"fn{j}")
                nc.vector.tensor_copy(fn_t[:], f_t[:])  # raw cast for margin dot
                fn.append(fn_t)
            for j in range(NB):
                f_t = f_ts[j]
                scr = ph0p.tile([P, D], dt.float32, tag="scr", bufs=2)
                fss = ph0p.tile([P, 1], dt.float32, tag=f"fss{j}")
                nc.vector.tensor_mul(scr[:], f_t[:], f_t[:])
                nc.vector.tensor_reduce(fss[:], scr[:],
                                        axis=mybir.AxisListType.X, op=Alu.add)
                invf = ph0p.tile([P, 1], dt.float32, tag=f"invf{j}")
                nc.scalar.activation(invf[:], fss[:], Act.Abs_reciprocal_sqrt,
                                     bias=0.0, scale=1.0 / (S_SCALE * S_SCALE))
                invfS.append(invf)

            # ---- remaining super-group-0 pieces ----
            for gi in range(1, SGG):
                glo = gi * GW
                ghi = min(sg0_hi - sg0_lo, (gi + 1) * GW)
                for k in range(NK):
                    nc.sync.dma_start(
                        wt_tiles[k][:, glo:ghi],
                        wt[k * P:(k + 1) * P, sg0_lo + glo:sg0_lo + ghi])
                    nc.vector.tensor_tensor(
                        out=sq_tiles[k][:, glo:ghi], in0=wt_tiles[k][:, glo:ghi],
                        in1=wt_tiles[k][:, glo:ghi], op=Alu.mult)
            sidx_t = constp.tile([P, NB], dt.int32, tag="sidx_t")
            nc.sync.dma_start(sidx_t[:], sidx[:])
            # ---- wg input DMAs (compute deferred past main loop) ----
            for j in range(NB):
                wg_t = ph0p.tile([P, D], dt.float32, tag=f"wg{j}")
                nc.sync.dma_start(wg_t[:], wg[j * P:(j + 1) * P, :])
                wg_ts.append(wg_t)

            # ---- margin values (deferred; DVE has slack mid-loop) ----
            val = []
            for j in range(NB):
                fn_t = fn[j]
                wg_t = wg_ts[j]
                scr2 = ph0p.tile([P, D], dt.float32, tag="scr", bufs=2)
                wss = ph0p.tile([P, 1], dt.float32, tag=f"wss{j}")
                nc.vector.tensor_mul(scr2[:], wg_t[:], wg_t[:])
                nc.vector.tensor_reduce(wss[:], scr2[:],
                                        axis=mybir.AxisListType.X, op=Alu.add)
                invwg = ph0p.tile([P, 1], dt.float32, tag=f"invwg{j}")
                nc.scalar.activation(invwg[:], wss[:], Act.Abs_reciprocal_sqrt,
                                     bias=0.0, scale=1.0)
                wgn_t = ph0p.tile([P, D], dt.bfloat16, tag=f"wgn{j}")
                nc.vector.tensor_scalar_mul(wgn_t[:], wg_t[:], invwg[:, 0:1])

                scr3 = ph0p.tile([P, D], dt.float32, tag="scr", bufs=2)
                dot = ph0p.tile([P, 1], dt.float32, tag=f"dot{j}")
                nc.vector.tensor_mul(scr3[:], fn_t[:], wgn_t[:])
                nc.vector.tensor_reduce(dot[:], scr3[:],
                                        axis=mybir.AxisListType.X, op=Alu.add)

                t = ph0p.tile([P, 1], dt.float32, tag=f"t{j}")
                nc.vector.tensor_scalar(
                    out=t[:], in0=dot[:], scalar1=invfS[j][:, 0:1],
                    scalar2=1.0 / S_SCALE, op0=Alu.mult, op1=Alu.mult)
                nc.vector.tensor_scalar_min(t[:], t[:], 1.0)
                nc.vector.tensor_scalar_max(t[:], t[:], -1.0)
                om = ph0p.tile([P, 1], dt.float32, tag=f"om{j}")
                nc.vector.tensor_mul(om[:], t[:], t[:])
                nc.vector.tensor_scalar(
                    out=om[:], in0=om[:], scalar1=-1.0, scalar2=1.0,
                    op0=Alu.mult, op1=Alu.add)
                rs = ph0p.tile([P, 1], dt.float32, tag=f"rs{j}")
                nc.scalar.activation(rs[:], om[:], Act.Abs_reciprocal_sqrt,
                                     bias=0.0, scale=1.0)
                r = ph0p.tile([P, 1], dt.float32, tag=f"r{j}")
                nc.vector.tensor_mul(r[:], om[:], rs[:])
                a1 = ph0p.tile([P, 1], dt.float32, tag=f"a1{j}")
                nc.vector.tensor_scalar_mul(a1[:], t[:], math.cos(M_MARGIN))
                a2 = ph0p.tile([P, 1], dt.float32, tag=f"a2{j}")
                nc.vector.tensor_scalar_mul(a2[:], r[:], math.sin(M_MARGIN))
                adjA = ph0p.tile([P, 1], dt.float32, tag=f"adjA{j}")
                nc.vector.tensor_tensor(out=adjA[:], in0=a1[:], in1=a2[:],
                                        op=Alu.subtract)
                mask = ph0p.tile([P, 1], dt.int8, tag=f"mask{j}")
                nc.vector.tensor_scalar(
                    out=mask[:], in0=t[:], scalar1=TH, scalar2=None, op0=Alu.is_gt)
                adj = ph0p.tile([P, 1], dt.float32, tag=f"adj{j}")
                nc.vector.tensor_scalar_sub(adj[:], t[:], MM_)  # on_false branch
                nc.vector.copy_predicated(adj[:], mask[:], adjA[:])
                val_t = ph0p.tile([P, 1],
                                  dt.float32 if host_scatter else dt.bfloat16,
                                  tag=f"val{j}")
                nc.vector.tensor_scalar_mul(val_t[:], adj[:], S_SCALE)
                val.append(val_t)

            # ---- main loop over class groups ----
            sg_lo = 0
            for g in range(NG):
                sg, gi = divmod(g, SGG)
                gw = _group_w(g)
                if gi == 0 and sg == 0:
                    sg_lo, sg_hi = _sg_bounds(0)
                elif gi == 0:
                    sg_lo, sg_hi = _sg_bounds(sg)
                    sgw = sg_hi - sg_lo
                    for k in range(NK):
                        wt_t = wtp.tile([P, SGG * GW], dt.bfloat16, tag=f"wt{k}",
                                        bufs=wtb)
                        nc.sync.dma_start(
                            wt_t[:, :sgw],
                            wt[k * P:(k + 1) * P, sg_lo:sg_hi])
                        sq_t = sqp.tile([P, SGG * GW], dt.bfloat16, tag=f"sq{k}",
                                        bufs=2)
                        nc.vector.tensor_tensor(
                            out=sq_t[:, :sgw], in0=wt_t[:, :sgw],
                            in1=wt_t[:, :sgw], op=Alu.mult)
                        wt_tiles[k] = wt_t
                        sq_tiles[k] = sq_t
                lo = g * GW - sg_lo  # offset within super-group tile

                # class inv-norms, replicated across partitions by the
                # all-ones stationary (same N-bound matmul cost)
                pn = ps_n.tile([P, GW], dt.float32, tag="pn")
                for k in range(NK):
                    nc.tensor.matmul(
                        pn[:, :gw], ones_sq[:], sq_tiles[k][:, lo:lo + gw],
                        start=(k == 0), stop=(k == NK - 1))
                invb = invp.tile([P, GW], dt.bfloat16, tag="invb")
                nc.scalar.activation(invb[:, :gw], pn[:, :gw],
                                     Act.Abs_reciprocal_sqrt, bias=0.0, scale=1.0)

                for j in range(NB):
                    po = ps_o.tile([P, GW], dt.float32, tag="po")
                    for k in range(NK):
                        nc.tensor.matmul(
                            po[:, :gw], fnT[k][:, j * P:(j + 1) * P],
                            wt_tiles[k][:, lo:lo + gw],
                            start=(k == 0), stop=(k == NK - 1))
                    oc = outp.tile([P, GW], dt.bfloat16, tag="oc")
                    nc.scalar.activation(oc[:, :gw], po[:, :gw], Act.Copy,
                                         bias=0.0, scale=invfS[j][:, 0:1])
                    ot = outp.tile([P, GW], dt.bfloat16, tag="ot")
                    nc.vector.tensor_tensor(
                        out=ot[:, :gw], in0=oc[:, :gw], in1=invb[:, :gw],
                        op=Alu.mult)
                    nc.sync.dma_start(
                        outs[j][:, g * GW:g * GW + gw], ot[:, :gw])

            # ---- margin scatter (per b-chunk tensor; WAW-ordered) ----
            if host_scatter:
                vcomb = ph0p.tile([P, NB], dt.float32, tag="vcomb")
                for j in range(NB):
                    nc.vector.tensor_copy(vcomb[:, j:j + 1], val[j][:, 0:1])
                nc.sync.dma_start(vout[:], vcomb[:])
            else:
                for j in range(NB):
                    nc.gpsimd.indirect_dma_start(
                        out=out_flats[j],
                        out_offset=bass.IndirectOffsetOnAxis(
                            ap=sidx_t[:, j:j + 1], axis=0),
                        in_=val[j][:, 0:1],
                        in_offset=None,
                        bounds_check=P * CS - 1,
                        oob_is_err=False,
                    )
    nc.compile()
    return nc


def _get_nc(opts=None):
    key = tuple(sorted((opts or {}).items()))
    if key not in _CACHE:
        _CACHE[key] = _build_nc(opts)
    return _CACHE[key]


def _enable_trace_hook():
    import sys
    import types
    import contextlib
    try:
        import antenv.axon_hooks  # noqa: F401
        return
    except ImportError:
        pass
    import antenv
    mod = types.ModuleType("antenv.axon_hooks")
    holder = [None]
    mod.set_axon_ntff_profile_hook = lambda h: holder.__setitem__(0, h)
    mod.get_axon_ntff_profile_hook = lambda: holder[0]
    sys.modules["antenv.axon_hooks"] = mod
    antenv.axon_hooks = mod
    try:
        from trn_agent_boot.trn_boot import _ntff_profile_via_ctypes
        mod.set_axon_ntff_profile_hook(
            _ntff_profile_via_ctypes("/opt/axon/libaxon_pjrt.so"))
    except Exception:
        pass


LAST_EXEC_NS = None
LAST_RESULTS = None
_OPTS = {}


def kernel(features, labels, weight):
    global LAST_EXEC_NS, LAST_RESULTS
    import ml_dtypes
    from concourse.bass_utils import run_bass_kernel_spmd

    features = np.ascontiguousarray(np.asarray(features), dtype=np.float32)
    weight = np.asarray(weight)
    labels = np.asarray(labels).astype(np.int64)

    trace = bool(int(os.environ.get("ARCFACE_TRACE", "0")))
    if trace:
        _enable_trace_hook()

    nc = _get_nc(_OPTS.get("opts"))

    wt_bf16 = weight.astype(ml_dtypes.bfloat16)
    featT_np = np.ascontiguousarray(features.T)
    wg_full = np.ascontiguousarray(weight[labels], dtype=np.float32)

    rows = np.arange(B, dtype=np.int64)
    in_maps = []
    for c in range(NCORES):
        c0 = c * CS
        wt_c = np.ascontiguousarray(wt_bf16[c0:c0 + CS].T)  # [D, CS] bf16
        lab_loc = labels - c0
        owned = (labels >= c0) & (labels < c0 + CS)
        prow = rows % P
        flat = np.where(owned, prow * CS + lab_loc, OOB).astype(np.int32)
        sidx_c = np.ascontiguousarray(flat.reshape(NB, P).T)  # [128, 4]
        in_maps.append({
            "wt": wt_c,
            "feat": features,
            "featT": featT_np,
            "wg": wg_full,
            "sidx": sidx_c,
        })

    res = run_bass_kernel_spmd(nc, in_maps, core_ids=list(range(NCORES)),
                               trace=trace)
    LAST_EXEC_NS = res.exec_time_ns
    LAST_RESULTS = res
    shards = [
        np.concatenate([res.results[c][f"out{j}"] for j in range(NB)], axis=0)
        for c in range(NCORES)
    ]
    full = np.concatenate(shards, axis=1).astype(np.float32)
    if (_OPTS.get("opts") or {}).get("host_scatter"):
        owner = (labels // CS).astype(np.int64)
        for c in range(NCORES):
            m = owner == c
            if not m.any():
                continue
            v = res.results[c]["vout"]  # [P, NB]
            b = rows[m]
            full[b, labels[m]] = v[b % P, b // P]
    return full


# revision 30
# speedup vs baseline: 1.0386x; 1.0386x over previous
"""ArcFace head on 8 TRN2 NeuronCores.

Class-parallel sharding: core c owns classes [c*12500, (c+1)*12500).
Each core computes out[b, c_local] = S * (F_hat_b . W_hat_c) for its class
shard, and fixes up the target column of rows whose label it owns with the
ArcFace margin via an indirect-DMA scatter.

Host-side prep is layout/indexing only: shard + transpose + bf16-cast of the
weight, a gather of weight rows by label (wg), and flat scatter indices. All
FLOPs (norms, matmul, margin trig) run on device.
"""

import math
import os

import numpy as np

B = 512
D = 512
C = 100000
NCORES = 8
CS = C // NCORES  # 12500 classes per core

M_MARGIN = 0.5
S_SCALE = 64.0
TH = math.cos(math.pi - M_MARGIN)
MM_ = math.sin(math.pi - M_MARGIN) * M_MARGIN
EPS = 1e-12

P = 128
NB = B // P            # 4 b-chunks
NK = D // P            # 4 k-chunks
GW = 512               # c-group width (psum free dim)
NG = (CS + GW - 1) // GW          # 25 groups, last = 212 wide
SGG = 5                # groups per super-group (DMA/square granularity)
NSG = (NG + SGG - 1) // SGG       # 5 super-groups

OOB = 2 ** 28          # scatter index sentinel for unowned rows

_CACHE = {}


def _group_w(g):
    return min(GW, CS - g * GW)


def _sg_bounds(sg):
    lo = sg * SGG * GW
    hi = min(CS, (sg + 1) * SGG * GW)
    return lo, hi


def _build_nc(opts=None):
    opts = opts or {}
    import concourse.tile as tile
    from concourse import bacc, mybir
    import concourse.bass as bass

    dt = mybir.dt
    Alu = mybir.AluOpType
    Act = mybir.ActivationFunctionType

    nc = bacc.Bacc("TRN2", target_bir_lowering=False, debug=False,
                   enable_asserts=False, num_devices=NCORES)

    wt = nc.dram_tensor("wt", [D, CS], dt.bfloat16, kind="ExternalInput").ap()
    feat = nc.dram_tensor("feat", [B, D], dt.float32, kind="ExternalInput").ap()
    featT = nc.dram_tensor("featT", [D, B], dt.float32, kind="ExternalInput").ap()
    wg = nc.dram_tensor("wg", [B, D], dt.float32, kind="ExternalInput").ap()
    sidx = nc.dram_tensor("sidx", [P, NB], dt.int32, kind="ExternalInput").ap()
    outs = [nc.dram_tensor(f"out{j}", [P, CS], dt.bfloat16,
                           kind="ExternalOutput").ap() for j in range(NB)]
    out_flats = [o.rearrange("b c -> (b c)")[:, None] for o in outs]
    host_scatter = bool(opts.get("host_scatter"))
    if host_scatter:
        vout = nc.dram_tensor("vout", [P, NB], dt.float32,
                              kind="ExternalOutput").ap()

    LN_S = math.log(S_SCALE)

    with tile.TileContext(nc) as tc:
        with (
            tc.tile_pool(name="const", bufs=1) as constp,
            tc.tile_pool(name="ph0", bufs=2) as ph0p,
            tc.tile_pool(name="wtp", bufs=8) as wtp,
            tc.tile_pool(name="sqp", bufs=8) as sqp,
            tc.tile_pool(name="invp", bufs=3) as invp,
            tc.tile_pool(name="outp", bufs=6) as outp,
            tc.tile_pool(name="ps_o", bufs=opts.get("ps_o", 6),
                         space="PSUM") as ps_o,
            tc.tile_pool(name="ps_n", bufs=opts.get("ps_n", 2),
                         space="PSUM") as ps_n,
        ):
            # ---- constants ----
            ones_sq = constp.tile([P, P], dt.bfloat16, tag="ones_sq")
            nc.vector.memset(ones_sq[:], 1.0)

            # ---- prefetch + square super-group 0, per-group pieces ----
            wt_tiles = [None] * NK
            sq_tiles = [None] * NK
            sg0_lo, sg0_hi = _sg_bounds(0)
            wtb = opts.get("wtbufs", 2)
            for k in range(NK):
                wt_tiles[k] = wtp.tile([P, SGG * GW], dt.bfloat16,
                                       tag=f"wt{k}", bufs=wtb, name=f"wt_t{k}")
                sq_tiles[k] = sqp.tile([P, SGG * GW], dt.bfloat16,
                                       tag=f"sq{k}", bufs=2, name=f"sq_t{k}")
            for gi in range(1):
                glo = gi * GW
                ghi = min(sg0_hi - sg0_lo, (gi + 1) * GW)
                for k in range(NK):
                    nc.sync.dma_start(
                        wt_tiles[k][:, glo:ghi],
                        wt[k * P:(k + 1) * P, sg0_lo + glo:sg0_lo + ghi])
                    nc.vector.tensor_tensor(
                        out=sq_tiles[k][:, glo:ghi], in0=wt_tiles[k][:, glo:ghi],
                        in1=wt_tiles[k][:, glo:ghi], op=Alu.mult)

            # ---- feature chain first: it gates the main matmuls ----
            fn = []
            fnT = []
            for k in range(NK):
                fnT_k = constp.tile([P, B], dt.bfloat16, tag=f"fnT{k}")
                fnT.append(fnT_k)
            f_ts = []
            wg_ts = []
            invfS = []  # S / |F_b| per b-chunk, folded into the out copy
            for k in range(NK):
                ft_f32 = ph0p.tile([P, B], dt.float32, tag=f"ftf{k}")
                nc.sync.dma_start(ft_f32[:], featT[k * P:(k + 1) * P, :])
                nc.vector.tensor_copy(fnT[k][:], ft_f32[:])  # f32 -> bf16
            for j in range(NB):
                f_t = ph0p.tile([P, D], dt.float32, tag=f"f{j}")
                nc.sync.dma_start(f_t[:], feat[j * P:(j + 1) * P, :])
                f_ts.append(f_t)
            for j in range(NB):
                f_t = f_ts[j]
                fn_t = ph0p.tile([P, D], dt.bfloat16, tag=f"fn{j}")
                nc.vector.tensor_copy(fn_t[:], f_t[:])  # raw cast for margin dot
                fn.append(fn_t)
            for j in range(NB):
                f_t = f_ts[j]
                scr = ph0p.tile([P, D], dt.float32, tag="scr", bufs=2)
                fss = ph0p.tile([P, 1], dt.float32, tag=f"fss{j}")
                nc.vector.tensor_mul(scr[:], f_t[:], f_t[:])
                nc.vector.tensor_reduce(fss[:], scr[:],
                                        axis=mybir.AxisListType.X, op=Alu.add)
                invf = ph0p.tile([P, 1], dt.float32, tag=f"invf{j}")
                nc.scalar.activation(invf[:], fss[:], Act.Abs_reciprocal_sqrt,
                                     bias=0.0, scale=1.0 / (S_SCALE * S_SCALE))
                invfS.append(invf)

            # ---- remaining super-group-0 pieces ----
            for gi in range(1, SGG):
                glo = gi * GW
                ghi = min(sg0_hi - sg0_lo, (gi + 1) * GW)
                for k in range(NK):
                    nc.sync.dma_start(
                        wt_tiles[k][:, glo:ghi],
                        wt[k * P:(k + 1) * P, sg0_lo + glo:sg0_lo + ghi])
                    nc.vector.tensor_tensor(
                        out=sq_tiles[k][:, glo:ghi], in0=wt_tiles[k][:, glo:ghi],
                        in1=wt_tiles[k][:, glo:ghi], op=Alu.mult)
            sidx_t = constp.tile([P, NB], dt.int32, tag="sidx_t")
            nc.sync.dma_start(sidx_t[:], sidx[:])
            # ---- wg input DMAs (compute deferred past main loop) ----
            for j in range(NB):
                wg_t = ph0p.tile([P, D], dt.float32, tag=f"wg{j}")
                nc.sync.dma_start(wg_t[:], wg[j * P:(j + 1) * P, :])
                wg_ts.append(wg_t)

            # ---- margin values (deferred; DVE has slack mid-loop) ----
            val = []
            for j in range(NB):
                fn_t = fn[j]
                wg_t = wg_ts[j]
                scr2 = ph0p.tile([P, D], dt.float32, tag="scr", bufs=2)
                wss = ph0p.tile([P, 1], dt.float32, tag=f"wss{j}")
                nc.vector.tensor_mul(scr2[:], wg_t[:], wg_t[:])
                nc.vector.tensor_reduce(wss[:], scr2[:],
                                        axis=mybir.AxisListType.X, op=Alu.add)
                invwg = ph0p.tile([P, 1], dt.float32, tag=f"invwg{j}")
                nc.scalar.activation(invwg[:], wss[:], Act.Abs_reciprocal_sqrt,
                                     bias=0.0, scale=1.0)
                wgn_t = ph0p.tile([P, D], dt.bfloat16, tag=f"wgn{j}")
                nc.vector.tensor_scalar_mul(wgn_t[:], wg_t[:], invwg[:, 0:1])

                scr3 = ph0p.tile([P, D], dt.float32, tag="scr", bufs=2)
                dot = ph0p.tile([P, 1], dt.float32, tag=f"dot{j}")
                nc.vector.tensor_mul(scr3[:], fn_t[:], wgn_t[:])
                nc.vector.tensor_reduce(dot[:], scr3[:],
                                        axis=mybir.AxisListType.X, op=Alu.add)

                t = ph0p.tile([P, 1], dt.float32, tag=f"t{j}")
                nc.vector.tensor_scalar(
                    out=t[:], in0=dot[:], scalar1=invfS[j][:, 0:1],
                    scalar2=1.0 / S_SCALE, op0=Alu.mult, op1=Alu.mult)
                nc.vector.tensor_scalar_min(t[:], t[:], 1.0)
                nc.vector.tensor_scalar_max(t[:], t[:], -1.0)
                om = ph0p.tile([P, 1], dt.float32, tag=f"om{j}")
                nc.vector.tensor_mul(om[:], t[:], t[:])
                nc.vector.tensor_scalar(
                    out=om[:], in0=om[:], scalar1=-1.0, scalar2=1.0,
                    op0=Alu.mult, op1=Alu.add)
                rs = ph0p.tile([P, 1], dt.float32, tag=f"rs{j}")
                nc.scalar.activation(rs[:], om[:], Act.Abs_reciprocal_sqrt,
                                     bias=0.0, scale=1.0)
                r = ph0p.tile([P, 1], dt.float32, tag=f"r{j}")
                nc.vector.tensor_mul(r[:], om[:], rs[:])
                a1 = ph0p.tile([P, 1], dt.float32, tag=f"a1{j}")
                nc.vector.tensor_scalar_mul(a1[:], t[:], math.cos(M_MARGIN))
                a2 = ph0p.tile([P, 1], dt.float32, tag=f"a2{j}")
                nc.vector.tensor_scalar_mul(a2[:], r[:], math.sin(M_MARGIN))
                adjA = ph0p.tile([P, 1], dt.float32, tag=f"adjA{j}")
                nc.vector.tensor_tensor(out=adjA[:], in0=a1[:], in1=a2[:],
                                        op=Alu.subtract)
                mask = ph0p.tile([P, 1], dt.int8, tag=f"mask{j}")
                nc.vector.tensor_scalar(
                    out=mask[:], in0=t[:], scalar1=TH, scalar2=None, op0=Alu.is_gt)
                adj = ph0p.tile([P, 1], dt.float32, tag=f"adj{j}")
                nc.vector.tensor_scalar_sub(adj[:], t[:], MM_)  # on_false branch
                nc.vector.copy_predicated(adj[:], mask[:], adjA[:])
                val_t = ph0p.tile([P, 1],
                                  dt.float32 if host_scatter else dt.bfloat16,
                                  tag=f"val{j}")
                nc.vector.tensor_scalar_mul(val_t[:], adj[:], S_SCALE)
                val.append(val_t)

            # ---- main loop over class groups ----
            sg_lo = 0
            for g in range(NG):
                sg, gi = divmod(g, SGG)
                gw = _group_w(g)
                if gi == 0 and sg == 0:
                    sg_lo, sg_hi = _sg_bounds(0)
                elif gi == 0:
                    sg_lo, sg_hi = _sg_bounds(sg)
                    sgw = sg_hi - sg_lo
                    h1 = min(2 * GW, sgw)  # first two groups land sooner
                    for k in range(NK):
                        wt_t = wtp.tile([P, SGG * GW], dt.bfloat16, tag=f"wt{k}",
                                        bufs=wtb)
                        sq_t = sqp.tile([P, SGG * GW], dt.bfloat16, tag=f"sq{k}",
                                        bufs=2)
                        nc.sync.dma_start(
                            wt_t[:, :h1],
                            wt[k * P:(k + 1) * P, sg_lo:sg_lo + h1])
                        nc.vector.tensor_tensor(
                            out=sq_t[:, :h1], in0=wt_t[:, :h1],
                            in1=wt_t[:, :h1], op=Alu.mult)
                        if h1 < sgw:
                            nc.sync.dma_start(
                                wt_t[:, h1:sgw],
                                wt[k * P:(k + 1) * P, sg_lo + h1:sg_hi])
                            nc.vector.tensor_tensor(
                                out=sq_t[:, h1:sgw], in0=wt_t[:, h1:sgw],
                                in1=wt_t[:, h1:sgw], op=Alu.mult)
                        wt_tiles[k] = wt_t
                        sq_tiles[k] = sq_t
                lo = g * GW - sg_lo  # offset within super-group tile

                # class inv-norms, replicated across partitions by the
                # all-ones stationary (same N-bound matmul cost)
                pn = ps_n.tile([P, GW], dt.float32, tag="pn")
                for k in range(NK):
                    nc.tensor.matmul(
                        pn[:, :gw], ones_sq[:], sq_tiles[k][:, lo:lo + gw],
                        start=(k == 0), stop=(k == NK - 1))
                invb = invp.tile([P, GW], dt.bfloat16, tag="invb")
                nc.scalar.activation(invb[:, :gw], pn[:, :gw],
                                     Act.Abs_reciprocal_sqrt, bias=0.0, scale=1.0)

                for j in range(NB):
                    po = ps_o.tile([P, GW], dt.float32, tag="po")
                    for k in range(NK):
                        nc.tensor.matmul(
                            po[:, :gw], fnT[k][:, j * P:(j + 1) * P],
                            wt_tiles[k][:, lo:lo + gw],
                            start=(k == 0), stop=(k == NK - 1))
                    oc = outp.tile([P, GW], dt.bfloat16, tag="oc")
                    nc.scalar.activation(oc[:, :gw], po[:, :gw], Act.Copy,
                                         bias=0.0, scale=invfS[j][:, 0:1])
                    ot = outp.tile([P, GW], dt.bfloat16, tag="ot")
                    nc.vector.tensor_tensor(
                        out=ot[:, :gw], in0=oc[:, :gw], in1=invb[:, :gw],
                        op=Alu.mult)
                    nc.sync.dma_start(
                        outs[j][:, g * GW:g * GW + gw], ot[:, :gw])

            # ---- margin scatter (per b-chunk tensor; WAW-ordered) ----
            if host_scatter:
                vcomb = ph0p.tile([P, NB], dt.float32, tag="vcomb")
                for j in range(NB):
                    nc.vector.tensor_copy(vcomb[:, j:j + 1], val[j][:, 0:1])
                nc.sync.dma_start(vout[:], vcomb[:])
            else:
                for j in range(NB):
                    nc.gpsimd.indirect_dma_start(
                        out=out_flats[j],
                        out_offset=bass.IndirectOffsetOnAxis(
                            ap=sidx_t[:, j:j + 1], axis=0),
                        in_=val[j][:, 0:1],
                        in_offset=None,
                        bounds_check=P * CS - 1,
                        oob_is_err=False,
                    )
    nc.compile()
    return nc


def _get_nc(opts=None):
    key = tuple(sorted((opts or {}).items()))
    if key not in _CACHE:
        _CACHE[key] = _build_nc(opts)
    return _CACHE[key]


def _enable_trace_hook():
    import sys
    import types
    import contextlib
    try:
        import antenv.axon_hooks  # noqa: F401
        return
    except ImportError:
        pass
    import antenv
    mod = types.ModuleType("antenv.axon_hooks")
    holder = [None]
    mod.set_axon_ntff_profile_hook = lambda h: holder.__setitem__(0, h)
    mod.get_axon_ntff_profile_hook = lambda: holder[0]
    sys.modules["antenv.axon_hooks"] = mod
    antenv.axon_hooks = mod
    try:
        from trn_agent_boot.trn_boot import _ntff_profile_via_ctypes
        mod.set_axon_ntff_profile_hook(
            _ntff_profile_via_ctypes("/opt/axon/libaxon_pjrt.so"))
    except Exception:
        pass


LAST_EXEC_NS = None
LAST_RESULTS = None
_OPTS = {}


def kernel(features, labels, weight):
    global LAST_EXEC_NS, LAST_RESULTS
    import ml_dtypes
    from concourse.bass_utils import run_bass_kernel_spmd

    features = np.ascontiguousarray(np.asarray(features), dtype=np.float32)
    weight = np.asarray(weight)
    labels = np.asarray(labels).astype(np.int64)

    trace = bool(int(os.environ.get("ARCFACE_TRACE", "0")))
    if trace:
        _enable_trace_hook()

    nc = _get_nc(_OPTS.get("opts"))

    wt_bf16 = weight.astype(ml_dtypes.bfloat16)
    featT_np = np.ascontiguousarray(features.T)
    wg_full = np.ascontiguousarray(weight[labels], dtype=np.float32)

    rows = np.arange(B, dtype=np.int64)
    in_maps = []
    for c in range(NCORES):
        c0 = c * CS
        wt_c = np.ascontiguousarray(wt_bf16[c0:c0 + CS].T)  # [D, CS] bf16
        lab_loc = labels - c0
        owned = (labels >= c0) & (labels < c0 + CS)
        prow = rows % P
        flat = np.where(owned, prow * CS + lab_loc, OOB).astype(np.int32)
        sidx_c = np.ascontiguousarray(flat.reshape(NB, P).T)  # [128, 4]
        in_maps.append({
            "wt": wt_c,
            "feat": features,
            "featT": featT_np,
            "wg": wg_full,
            "sidx": sidx_c,
        })

    res = run_bass_kernel_spmd(nc, in_maps, core_ids=list(range(NCORES)),
                               trace=trace)
    LAST_EXEC_NS = res.exec_time_ns
    LAST_RESULTS = res
    shards = [
        np.concatenate([res.results[c][f"out{j}"] for j in range(NB)], axis=0)
        for c in range(NCORES)
    ]
    full = np.concatenate(shards, axis=1).astype(np.float32)
    if (_OPTS.get("opts") or {}).get("host_scatter"):
        owner = (labels // CS).astype(np.int64)
        for c in range(NCORES):
            m = owner == c
            if not m.any():
                continue
            v = res.results[c]["vout"]  # [P, NB]
            b = rows[m]
            full[b, labels[m]] = v[b % P, b // P]
    return full
